# revision 3
# baseline (speedup 1.0000x reference)
"""Int32 3x3 conv2d (stride 1, pad 1) as bf16 matmuls on 8 TRN2 cores.

Problem: x[16,256,56,56] (*) w[256,256,3,3] + b[256] -> y[16,256,56,56],
all int32, values in [0,127).

Trick: values 0..126 are exactly representable in bf16, every product is
an integer < 2^14, and every accumulation stays < 2^24, so a bf16 matmul
with fp32 PSUM accumulation produces bit-exact integer results.

Layout: each image is zero-padded to 58x58 and flattened (stride-58 rows,
+4 slack elements so shifted reads never run off the buffer). The 3x3 conv
becomes 9 shifted [Cin,Cout]^T @ [Cin,pixels] matmuls accumulated in PSUM.
Output pixel chunks of 464 = 8 rows x 58 cols fit one PSUM bank; the two
garbage columns per row (w=56,57) are skipped by the output DMA.

Sharding: data-parallel over batch, 2 images per core; weights replicated.
"""

import numpy as np
import ml_dtypes

B, C, H, W = 16, 256, 56, 56
HP, WP = H + 2, W + 2          # 58, 58 padded
IMG = HP * WP                  # 3364 flat padded image
IMG_PAD = IMG + 4              # 3368, slack for shifted reads
N_CORES = 8
IMG_PER_CORE = B // N_CORES    # 2
CHUNK = 464                    # 8 output rows x 58 cols, fits one PSUM bank
N_CHUNKS = (H * WP) // CHUNK   # 7
ROWS_PER_CHUNK = CHUNK // WP   # 8

_BF16 = ml_dtypes.bfloat16


def _build_program():
    import concourse.bass as bass
    import concourse.mybir as mybir
    from concourse import bacc
    from concourse.tile import TileContext

    nc = bacc.Bacc("TRN2", target_bir_lowering=False, debug=False)

    x_h = nc.dram_tensor(
        "x", [2, 128, IMG_PER_CORE * IMG_PAD], mybir.dt.bfloat16,
        kind="ExternalInput",
    )
    w_h = nc.dram_tensor(
        "w", [128, 2 * 2 * 9 * 128], mybir.dt.bfloat16, kind="ExternalInput"
    )
    b_h = nc.dram_tensor("b", [128, 2], mybir.dt.float32, kind="ExternalInput")
    y_h = nc.dram_tensor(
        "y", [IMG_PER_CORE, 2, 128, H, W], mybir.dt.int32, kind="ExternalOutput"
    )

    with TileContext(nc) as tc:
        with (
            tc.tile_pool(name="const", bufs=1) as const_pool,
            tc.tile_pool(name="xin", bufs=1) as x_pool,
            tc.tile_pool(name="psum", bufs=8, space="PSUM") as psum_pool,
            tc.tile_pool(name="outs", bufs=6) as out_pool,
        ):
            w_sb = const_pool.tile([128, 2 * 2 * 9 * 128], mybir.dt.bfloat16)
            nc.sync.dma_start(w_sb[:, :], w_h.ap())
            b_sb = const_pool.tile([128, 2], mybir.dt.float32)
            nc.sync.dma_start(b_sb[:, :], b_h.ap())

            # one x tile per (ci_chunk, img) so matmuls only wait on the
            # slice they read
            x_sb = {}
            for ci in range(2):
                for img in range(IMG_PER_CORE):
                    t = x_pool.tile([128, IMG_PAD], mybir.dt.bfloat16,
                                    tag=f"x_{ci}_{img}")
                    nc.sync.dma_start(
                        t[:, :],
                        x_h.ap()[ci][:, img * IMG_PAD:(img + 1) * IMG_PAD],
                    )
                    x_sb[ci, img] = t

            for img in range(IMG_PER_CORE):
                for co in range(2):
                    for pc in range(N_CHUNKS):
                        ps = psum_pool.tile([128, CHUNK], mybir.dt.float32)
                        n_mm = 0
                        for ci in range(2):
                            for k in range(9):
                                kh, kw = divmod(k, 3)
                                off = kh * WP + kw + pc * CHUNK
                                lhsT = w_sb[
                                    :,
                                    ((ci * 2 + co) * 9 + k) * 128:
                                    ((ci * 2 + co) * 9 + k + 1) * 128,
                                ]
                                rhs = x_sb[ci, img][:, off:off + CHUNK]
                                nc.tensor.matmul(
                                    ps[:, :], lhsT, rhs,
                                    start=(n_mm == 0), stop=(n_mm == 17),
                                )
                                n_mm += 1
                        ot = out_pool.tile([128, CHUNK], mybir.dt.int32)
                        nc.vector.tensor_scalar_add(
                            ot[:, :], ps[:, :], b_sb[:, co:co + 1]
                        )
                        src = ot[:, :].rearrange("p (r c) -> p r c", c=WP)
                        dst = y_h.ap()[img, co][
                            :, pc * ROWS_PER_CHUNK:(pc + 1) * ROWS_PER_CHUNK, :
                        ]
                        nc.sync.dma_start(dst, src[:, :, 0:W])

    nc.compile()
    return nc


_NC = None
LAST_RESULT = None  # BassKernelResults of the most recent run (for harnesses)


def kernel(x_int: np.ndarray, weight_int: np.ndarray, bias_int: np.ndarray):
    from concourse.bass_utils import run_bass_kernel_spmd

    global _NC, LAST_RESULT
    if _NC is None:
        _NC = _build_program()
    nc = _NC

    x_int = np.asarray(x_int)
    weight_int = np.asarray(weight_int)
    bias_int = np.asarray(bias_int)

    # x: pad to 58x58 (+4 slack), cast to bf16, split channels into two
    # 128-partition chunks: [ci_chunk, 128, img, IMG_PAD] per core.
    x_pad = np.zeros((B, C, HP, WP), dtype=_BF16)
    x_pad[:, :, 1:57, 1:57] = x_int.astype(_BF16)
    x_flat = x_pad.reshape(B, 2, 128, IMG)

    # w[co,ci,kh,kw] -> [ci_part, ci_chunk, co_chunk, k, co_part]
    w_t = (
        weight_int.astype(_BF16)
        .reshape(2, 128, 2, 128, 9)          # [co_c, co_p, ci_c, ci_p, k]
        .transpose(3, 2, 0, 4, 1)            # [ci_p, ci_c, co_c, k, co_p]
        .reshape(128, 2 * 2 * 9 * 128)
    )
    # note: weight_int.reshape(2,128,2,128,9) splits [256,256,3,3] as
    # [co_chunk, co_part, ci_chunk, ci_part, kh*kw]
    b_t = np.ascontiguousarray(
        bias_int.astype(np.float32).reshape(2, 128).T
    )

    in_maps = []
    for c in range(N_CORES):
        xc = np.zeros((2, 128, IMG_PER_CORE, IMG_PAD), dtype=_BF16)
        for img in range(IMG_PER_CORE):
            # x_flat[b, ci_chunk, ci_part, flat] -> [ci_chunk, part, img, :]
            xc[:, :, img, :IMG] = x_flat[c * IMG_PER_CORE + img]
        in_maps.append(
            {
                "x": xc.reshape(2, 128, IMG_PER_CORE * IMG_PAD),
                "w": w_t,
                "b": b_t,
            }
        )

    res = run_bass_kernel_spmd(nc, in_maps, core_ids=list(range(N_CORES)))
    LAST_RESULT = res

    y = np.empty((B, C, H, W), dtype=np.int32)
    for c in range(N_CORES):
        yc = res.results[c]["y"]  # [img, co_chunk, 128, H, W]
        for img in range(IMG_PER_CORE):
            y[c * IMG_PER_CORE + img] = yc[img].reshape(C, H, W)
    return y


# revision 4
# speedup vs baseline: 1.0331x; 1.0331x over previous
"""Int32 3x3 conv2d (stride 1, pad 1) as bf16 matmuls on 8 TRN2 cores.

Problem: x[16,256,56,56] (*) w[256,256,3,3] + b[256] -> y[16,256,56,56],
all int32, values in [0,127).

Trick: values 0..126 are exactly representable in bf16, every product is
an integer < 2^14, and every accumulation stays < 2^24, so a bf16 matmul
with fp32 PSUM accumulation produces bit-exact integer results.

Layout: each image is zero-padded to 58x58. The 3x3 conv becomes 9
shifted [Cin,Cout]^T @ [Cin,pixels] matmuls accumulated in PSUM; pixel
tiles are 8 output rows x 56 cols = 448 columns (one PSUM bank), read
from the padded image through a strided access pattern so only valid
pixels are computed.

Sharding: data-parallel over batch, 2 images per core; weights replicated.
"""

import numpy as np
import ml_dtypes

B, C, H, W = 16, 256, 56, 56
HP, WP = H + 2, W + 2          # 58, 58 padded
IMG = HP * WP                  # 3364 flat padded image
N_CORES = 8
IMG_PER_CORE = B // N_CORES    # 2
ROWS_PER_CHUNK = 8
CHUNK = ROWS_PER_CHUNK * W     # 448 valid pixels, fits one PSUM bank
N_CHUNKS = H // ROWS_PER_CHUNK  # 7
N_WARM = 9                     # cold matmuls to flip the HAM clock gate

_BF16 = ml_dtypes.bfloat16


def _build_program():
    import concourse.bass as bass
    import concourse.mybir as mybir
    from concourse import bacc
    from concourse.tile import TileContext

    nc = bacc.Bacc("TRN2", target_bir_lowering=False, debug=False)

    x_h = nc.dram_tensor(
        "x", [2, 128, IMG_PER_CORE * IMG], mybir.dt.bfloat16,
        kind="ExternalInput",
    )
    w_h = nc.dram_tensor(
        "w", [128, 2 * 2 * 9 * 128], mybir.dt.bfloat16, kind="ExternalInput"
    )
    b_h = nc.dram_tensor("b", [128, 2], mybir.dt.float32, kind="ExternalInput")
    y_h = nc.dram_tensor(
        "y", [IMG_PER_CORE, 2, 128, H, W], mybir.dt.int32, kind="ExternalOutput"
    )

    with TileContext(nc) as tc:
        with (
            tc.tile_pool(name="const", bufs=1) as const_pool,
            tc.tile_pool(name="xin", bufs=1) as x_pool,
            tc.tile_pool(name="psum", bufs=7, space="PSUM") as psum_pool,
            tc.tile_pool(name="warm", bufs=1, space="PSUM") as warm_pool,
            tc.tile_pool(name="outs", bufs=6) as out_pool,
        ):
            # PE warm-up: ~3.4us of junk matmuls on a zeroed tile while the
            # input DMAs land, so the HAM clock gate is at 8/8 (2.4 GHz)
            # when the real matmuls start.
            wz = const_pool.tile([128, 128 + CHUNK], mybir.dt.bfloat16)
            nc.vector.memset(wz[:, :], 0.0)
            wps = warm_pool.tile([128, CHUNK], mybir.dt.float32)
            for i in range(N_WARM):
                nc.tensor.matmul(
                    wps[:, :], wz[:, 0:128], wz[:, 128:128 + CHUNK],
                    start=True, stop=True,
                )

            # weights: one DMA per (ci_chunk, co_chunk) slice, first-needed
            # first, so the first real matmul group starts ASAP
            w_sb = const_pool.tile([128, 2 * 2 * 9 * 128], mybir.dt.bfloat16)

            def w_sl(ci, co):
                s = (ci * 2 + co) * 9 * 128
                return slice(s, s + 9 * 128)

            for ci, co in ((0, 0), (1, 0), (0, 1), (1, 1)):
                nc.sync.dma_start(
                    w_sb[:, w_sl(ci, co)], w_h.ap()[:, w_sl(ci, co)]
                )
            b_sb = const_pool.tile([128, 2], mybir.dt.float32)
            nc.sync.dma_start(b_sb[:, :], b_h.ap())

            # one x tile per (ci_chunk, img); viewed [128, 58, 58]
            x_sb = {}
            for img in range(IMG_PER_CORE):
                for ci in range(2):
                    t = x_pool.tile([128, IMG], mybir.dt.bfloat16,
                                    tag=f"x_{ci}_{img}")
                    nc.sync.dma_start(
                        t[:, :],
                        x_h.ap()[ci][:, img * IMG:(img + 1) * IMG],
                    )
                    x_sb[ci, img] = t[:, :].rearrange("p (r c) -> p r c", c=WP)

            for img in range(IMG_PER_CORE):
                for co in range(2):
                    for pc in range(N_CHUNKS):
                        r0 = pc * ROWS_PER_CHUNK
                        ps = psum_pool.tile([128, CHUNK], mybir.dt.float32)
                        n_mm = 0
                        for ci in range(2):
                            for k in range(9):
                                kh, kw = divmod(k, 3)
                                lhsT = w_sb[
                                    :,
                                    ((ci * 2 + co) * 9 + k) * 128:
                                    ((ci * 2 + co) * 9 + k + 1) * 128,
                                ]
                                rhs = x_sb[ci, img][
                                    :, r0 + kh:r0 + kh + ROWS_PER_CHUNK,
                                    kw:kw + W,
                                ]
                                nc.tensor.matmul(
                                    ps[:, :], lhsT, rhs,
                                    start=(n_mm == 0), stop=(n_mm == 17),
                                )
                                n_mm += 1
                        ot = out_pool.tile([128, CHUNK], mybir.dt.int32)
                        nc.vector.tensor_scalar_add(
                            ot[:, :], ps[:, :], b_sb[:, co:co + 1]
                        )
                        dst = y_h.ap()[img, co].rearrange("p h w -> p (h w)")[
                            :, pc * CHUNK:(pc + 1) * CHUNK
                        ]
                        nc.sync.dma_start(dst, ot[:, :])

    nc.compile()
    return nc


_NC = None
LAST_RESULT = None  # BassKernelResults of the most recent run (for harnesses)


def kernel(x_int: np.ndarray, weight_int: np.ndarray, bias_int: np.ndarray):
    from concourse.bass_utils import run_bass_kernel_spmd

    global _NC, LAST_RESULT
    if _NC is None:
        _NC = _build_program()
    nc = _NC

    x_int = np.asarray(x_int)
    weight_int = np.asarray(weight_int)
    bias_int = np.asarray(bias_int)

    # x: pad to 58x58, cast to bf16, split channels into two 128-partition
    # chunks: per core [ci_chunk, 128, img, IMG].
    x_pad = np.zeros((B, C, HP, WP), dtype=_BF16)
    x_pad[:, :, 1:57, 1:57] = x_int.astype(_BF16)
    x_flat = x_pad.reshape(B, 2, 128, IMG)

    # w[co,ci,kh,kw] -> [ci_part, ci_chunk, co_chunk, k, co_part]
    w_t = (
        weight_int.astype(_BF16)
        .reshape(2, 128, 2, 128, 9)          # [co_c, co_p, ci_c, ci_p, k]
        .transpose(3, 2, 0, 4, 1)            # [ci_p, ci_c, co_c, k, co_p]
        .reshape(128, 2 * 2 * 9 * 128)
    )
    w_t = np.ascontiguousarray(w_t)
    b_t = np.ascontiguousarray(
        bias_int.astype(np.float32).reshape(2, 128).T
    )

    in_maps = []
    for c in range(N_CORES):
        xc = np.ascontiguousarray(
            x_flat[c * IMG_PER_CORE:(c + 1) * IMG_PER_CORE].transpose(1, 2, 0, 3)
        )  # [ci_chunk, 128, img, IMG]
        in_maps.append(
            {
                "x": xc.reshape(2, 128, IMG_PER_CORE * IMG),
                "w": w_t,
                "b": b_t,
            }
        )

    res = run_bass_kernel_spmd(nc, in_maps, core_ids=list(range(N_CORES)))
    LAST_RESULT = res

    y = np.empty((B, C, H, W), dtype=np.int32)
    for c in range(N_CORES):
        yc = res.results[c]["y"]  # [img, co_chunk, 128, H, W]
        for img in range(IMG_PER_CORE):
            y[c * IMG_PER_CORE + img] = yc[img].reshape(C, H, W)
    return y
